# revision 28
# baseline (speedup 1.0000x reference)
"""Trainium2 Bass kernel: causal GQA attention (fp16 pipeline).

Problem: B=2, Sq=Sk=2048, H=32, Hkv=8, D=128, fp32 in/out, causal +
key-padding mask.

Sharding (8 cores): head-parallel. Core c takes q-heads [4c, 4c+4) for both
batches; those 4 heads share exactly one kv head (c) per batch, so each core
runs 8 independent (batch, head) pairs — K/V loaded once per batch, no comms.

All device data is fp16 (host converts): matmuls run at 1 PE cycle/row with
no minimum free-dim width, DVE elementwise ops get the 2-byte perf modes,
and DMA bytes halve. Measured numerics: ~4e-4 rel err vs the fp32 reference
(tolerance 2e-2).

Per (batch, head) pair the scores are built TRANSPOSED (keys on partitions,
queries on free) in 512-query groups, key chunks of 128 in 3-chunk ST tiles
(PSUM [128, 3, 512] = 3 banks x 2 bufs + 2 O^T banks = 8 banks). Strict
engine separation so the bottleneck ACT engine never waits on anything but
the PE:

  PE  : S^T[j] = K_j @ Q_g^T, exact causal trimming (live cols only);
        diag chunks += I.T @ tri (128-wide fp16 bias matmul);
        O^T += V_j^T @ P^T[j] accumulated over the group.
        Order is software-pipelined: QK(t0) QK(t1) PV(t0) QK(t2) PV(t1)...
  ACT : P^T = exp(scale*S^T) per ST tile, one fused instruction into a
        per-group P tile [128, 16, 512] fp16.
  DVE : zero the stale cols of diagonal chunks (memset), then one pairwise
        fold-tree per group (tensor_tensor adds at the 2x 2-byte rate over
        from-0 slices) producing acc[128,512] = sum of P over the group's
        key chunks.
  Pool: rb = partition_all_reduce(acc) (cross-partition softmax denom),
        out = O^T / rb (single tensor_tensor divide, PSUM read), then the
        output DMA on the same queue — no cross-engine ping-pong.
  SP  : all input DMAs (sync ring).

The key-padding mask folds into the exp bias per key chunk (bias operand
indexes partitions = keys). The all-ones-mask fast path (the spec's fill)
uses fused tile exps; a non-trivial mask falls back to per-chunk biases.
"""

import math
import sys

import numpy as np

for _p in ("/opt/trn_rl_repo",):
    if _p not in sys.path:
        sys.path.append(_p)

import concourse.bass as bass
import concourse.tile as tile
from concourse import bacc, bass_isa, mybir
from concourse.bass import ts
from concourse.bass_utils import run_bass_kernel_spmd

B = 2
S = 2048
H = 32
HKV = 8
D = 128
N_CORES = 8
HPC = H // N_CORES  # q heads per core = 4
PAIRS = B * HPC  # 8 (batch, head) pairs per core
NG = S // 512  # 4 q-groups of 512 per pair
NCHUNK = S // 128  # 16 key chunks of 128
SCALE = 1.0 / math.sqrt(D)
NEG = -10000.0
TCH = 2  # key chunks per ST tile

# normalize on Pool reading O^T straight from PSUM (False: DVE rcp+mul path)
POOL_DIV = True

F32 = mybir.dt.float32
F16 = mybir.dt.float16
EXP = mybir.ActivationFunctionType.Exp
ADD = mybir.AluOpType.add
DIVIDE = mybir.AluOpType.divide


def build_module(uniform_mask: bool = True):
    nc = bacc.Bacc("TRN2", target_bir_lowering=False, debug=False, num_devices=1)

    qt = nc.dram_tensor("qt", [PAIRS, D, S], F16, kind="ExternalInput").ap()
    kt = nc.dram_tensor("kt", [B, D, S], F16, kind="ExternalInput").ap()
    v = nc.dram_tensor("v", [B, S, D], F16, kind="ExternalInput").ap()
    tri = nc.dram_tensor("tri", [D, 640], F16, kind="ExternalInput").ap()
    pb = nc.dram_tensor("pb", [B, S], F32, kind="ExternalInput").ap()
    ot = nc.dram_tensor("ot", [PAIRS, NG, D, 512], F16, kind="ExternalOutput").ap()

    with tile.TileContext(nc) as tc:
        with (
            tc.tile_pool(name="consts", bufs=1) as consts,
            tc.tile_pool(name="kv", bufs=2) as kv_pool,
            tc.tile_pool(name="q", bufs=4) as q_pool,
            tc.tile_pool(name="ptg", bufs=3) as ptg_pool,
            tc.tile_pool(name="fold", bufs=2) as fold_pool,
            tc.tile_pool(name="acc", bufs=3) as acc_pool,
            tc.tile_pool(name="rb", bufs=3) as rb_pool,
            tc.tile_pool(name="osb", bufs=2) as osb_pool,
            tc.tile_pool(name="o32", bufs=3) as o32_pool,
            tc.tile_pool(name="st_ps", bufs=3, space="PSUM") as st_pool,
            tc.tile_pool(name="ot_ps", bufs=2, space="PSUM") as ot_pool,
        ):
            trid_sb = consts.tile([D, 640], F16)
            negtri_sb = trid_sb[:, :512]  # [neg | tri]: col 512-w.. masks w cols
            ident_sb = trid_sb[:, 512:]
            # warm the ACT exp table during the initial DMAs
            warm_in = consts.tile([1, 2], F32)
            nc.vector.memset(warm_in[:], 1.0)
            warm = consts.tile([1, 2], F32)
            nc.scalar.activation(warm[:], warm_in[:], EXP, scale=1.0)
            for _i in range(3):
                st = st_pool.tile([D, TCH, 512], F32)
                nc.vector.memset(st[:], 0.0)

            kvs = {}
            qts = {}

            def _load_inputs(pairs):
                """Issue input DMAs. Few, large transfers: every DMA burns a
                HWDGE gen slot (625ns, serial device) and a completion-sem
                slot (8 HW slots; reuse blocks until the prior user's waiters
                ran, which throttles prefetch depth). Pair 0 splits its first
                slices out so compute starts ASAP."""
                for p in pairs:
                    b = p // HPC
                    first = p == 0
                    if first:
                        # ordered for fastest time-to-first-matmul
                        kt_sb = kv_pool.tile([D, S], F16, tag="kt")
                        v_r = v[b].rearrange("(j k) d -> k j d", k=128)
                        v_sb = kv_pool.tile([D, NCHUNK, D], F16, tag="v")
                        pb_sb = kv_pool.tile([D, NCHUNK], F32, tag="pb")
                        kvs[b] = (kt_sb, v_sb, pb_sb)
                        qt_sb = q_pool.tile([D, S], F16, tag="qt")
                        qts[p] = qt_sb
                        nc.sync.dma_start(kt_sb[:, ts(0, 512)], kt[b][:, ts(0, 512)])
                        nc.sync.dma_start(qt_sb[:, ts(0, 512)], qt[p][:, ts(0, 512)])
                        nc.sync.dma_start(trid_sb[:], tri[:])
                        nc.sync.dma_start(v_sb[:, ts(0, 4), :], v_r[:, ts(0, 4), :])
                        nc.sync.dma_start(kt_sb[:, ts(1, 512)], kt[b][:, ts(1, 512)])
                        nc.sync.dma_start(qt_sb[:, ts(1, 512)], qt[p][:, ts(1, 512)])
                        nc.sync.dma_start(kt_sb[:, 1024:], kt[b][:, 1024:])
                        nc.sync.dma_start(qt_sb[:, 1024:], qt[p][:, 1024:])
                        nc.sync.dma_start(v_sb[:, 4:, :], v_r[:, 4:, :])
                        if not uniform_mask:
                            nc.sync.dma_start(
                                pb_sb[:], pb[b].rearrange("(j k) -> k j", k=128)
                            )
                        continue
                    if p % HPC == 0 and b not in kvs:
                        kt_sb = kv_pool.tile([D, S], F16, tag="kt")
                        v_r = v[b].rearrange("(j k) d -> k j d", k=128)
                        v_sb = kv_pool.tile([D, NCHUNK, D], F16, tag="v")
                        pb_sb = kv_pool.tile([D, NCHUNK], F32, tag="pb")
                        kvs[b] = (kt_sb, v_sb, pb_sb)
                        nc.sync.dma_start(kt_sb[:], kt[b][:])
                        nc.sync.dma_start(v_sb[:], v_r[:])
                        if not uniform_mask:
                            nc.sync.dma_start(
                                pb_sb[:], pb[b].rearrange("(j k) -> k j", k=128)
                            )
                    qt_sb = q_pool.tile([D, S], F16, tag="qt")
                    qts[p] = qt_sb
                    nc.sync.dma_start(qt_sb[:], qt[p][:])

            # global tile stream: smallest group first on the first pair
            # (fast ramp), largest first on the last pair (short drain tail)
            stream = []
            for pair in range(PAIRS):
                groups = (
                    list(reversed(range(NG))) if pair == PAIRS - 1 else range(NG)
                )
                for g in groups:
                    nj = 4 * (g + 1)
                    ntile = (nj + TCH - 1) // TCH
                    tiles_g = [
                        list(range(TCH * t, min(TCH * t + TCH, nj)))
                        for t in range(ntile)
                    ]
                    # emit the diagonal tiles first and a full-width tile
                    # last, so the group's final exp is big enough to cover
                    # the next group's QK latency (the first emitted tile
                    # still starts at a qlo=0 chunk for the PSUM reset)
                    d0 = (4 * g) // TCH
                    tiles_g = tiles_g[d0:] + tiles_g[:d0]
                    for t, chunks in enumerate(tiles_g):
                        stream.append(
                            dict(
                                pair=pair,
                                g=g,
                                nj=nj,
                                chunks=chunks,
                                qlos=[max(0, 128 * (j - 4 * g)) for j in chunks],
                                first=(t == 0),
                                last=(t == ntile - 1),
                            )
                        )
            # prefetch points: issue pair p's qt (and batch kv) when the
            # previous pair enters its last group
            prefetch_at = {0: [0]}
            for i, tl in enumerate(stream):
                if tl["pair"] > 0 and (
                    i == 0 or stream[i - 1]["pair"] != tl["pair"]
                ):
                    # find start of previous pair's last group
                    j = i - 1
                    while j > 0 and stream[j - 1]["g"] == stream[i - 1]["g"] and (
                        stream[j - 1]["pair"] == stream[i - 1]["pair"]
                    ):
                        j -= 1
                    prefetch_at.setdefault(j, []).append(tl["pair"])

            gstate = {}  # (pair, g) -> dict(ot_ps, pt_g)
            osbs = {}  # pair -> out_sb tile [D, NG, 512]
            done_groups = {}  # pair -> count of completed groups

            def _emit_qk(tl):
                pair, g = tl["pair"], tl["g"]
                kt_sb = kvs[pair // HPC][0]
                qt_sb = qts[pair]
                if (pair, g) not in gstate:
                    gstate[(pair, g)] = dict(
                        ot_ps=ot_pool.tile([D, 512], F32, name="ot_acc"),
                        pt_g=ptg_pool.tile([D, NCHUNK, 512], F16, name="pt_g"),
                    )
                st = st_pool.tile([D, TCH, 512], F32)
                tl["st"] = st
                qmin = min(tl["qlos"])
                for idx, j in enumerate(tl["chunks"]):
                    u = j - 4 * g
                    qlo = tl["qlos"][idx]
                    nc.tensor.matmul(
                        st[:, idx, qlo:],
                        lhsT=kt_sb[:, ts(j, 128)],
                        rhs=qt_sb[:, g * 512 + qlo : (g + 1) * 512],
                        start=True,
                        stop=(u < 0),
                    )
                # mask matmuls after all plain QKs: keeps the tri-const DMA
                # off the first-QK critical path (PSUM adds commute)
                for idx, j in enumerate(tl["chunks"]):
                    u = j - 4 * g
                    qlo = tl["qlos"][idx]
                    if u >= 0:
                        # causal bias on the PE itself; extended down to qmin
                        # so the fused exp writes exact zeros over the stale
                        # sub-diagonal cols (no DVE cleanup needed)
                        w = qlo + 128 - qmin
                        nc.tensor.matmul(
                            st[:, idx, qmin : qlo + 128],
                            lhsT=ident_sb[:],
                            rhs=negtri_sb[:, 512 - w :],
                            start=False,
                            stop=True,
                        )

            def _emit_exp(tl):
                pair, g = tl["pair"], tl["g"]
                pt_g = gstate[(pair, g)]["pt_g"]
                pb_sb = kvs[pair // HPC][2]
                chunks, st = tl["chunks"], tl["st"]
                nch = len(chunks)
                j0 = chunks[0]
                if uniform_mask:
                    qmin = min(tl["qlos"])
                    nc.scalar.activation(
                        pt_g[:, j0 : j0 + nch, qmin:],
                        st[:, :nch, qmin:],
                        EXP,
                        scale=SCALE,
                    )
                else:
                    for idx, j in enumerate(chunks):
                        qlo = tl["qlos"][idx]
                        nc.scalar.activation(
                            pt_g[:, j, qlo:],
                            st[:, idx, qlo:],
                            EXP,
                            bias=pb_sb[:, j : j + 1],
                            scale=SCALE,
                        )
                # zero the cols below qmin that this tile's exp never writes
                # (the extended mask only yields exp=0 down to qmin) so the
                # full-width fold-tree sums exact zeros there
                for idx, j in enumerate(chunks):
                    qlo = qmin if uniform_mask else tl["qlos"][idx]
                    if qlo > 0:
                        nc.vector.memset(pt_g[:, j, :qlo], 0.0)

            def _emit_fold(tl):
                # pairwise fold-tree on DVE: acc = sum of P over the group's
                # chunks. Depends only on the exps, so it is emitted right
                # after the group's last exp to keep the DVE queue flowing.
                pair, g, nj = tl["pair"], tl["g"], tl["nj"]
                gs = gstate[(pair, g)]
                acc = acc_pool.tile([D, 512], F16)
                gs["acc"] = acc
                with nc.allow_low_precision(
                    reason="fp16 softmax denominator, consistent with the "
                    "fp16 P used in PV; pairwise tree, ~1e-3 rel"
                ):
                    n = nj
                    src = gs["pt_g"]
                    while n > 1:
                        half = n // 2
                        if n == 2:
                            nc.vector.tensor_tensor(
                                acc[:], src[:, 0], src[:, 1], ADD
                            )
                            n = 1
                        elif n % 2 == 0:
                            dst = fold_pool.tile(
                                [D, half, 512], F16, tag=f"f{half}"
                            )
                            nc.vector.tensor_tensor(
                                dst[:], src[:, :half], src[:, half : 2 * half], ADD
                            )
                            src, n = dst, half
                        else:  # n == 3
                            dst = fold_pool.tile([D, 512], F16, tag="f1")
                            nc.vector.tensor_tensor(
                                dst[:], src[:, 0], src[:, 1], ADD
                            )
                            nc.vector.tensor_tensor(acc[:], dst[:], src[:, 2], ADD)
                            n = 1

            def _emit_pv(tl):
                pair, g, nj = tl["pair"], tl["g"], tl["nj"]
                gs = gstate[(pair, g)]
                v_sb = kvs[pair // HPC][1]
                nch = len(tl["chunks"])
                for idx, j in enumerate(tl["chunks"]):
                    qlo = tl["qlos"][idx]
                    nc.tensor.matmul(
                        gs["ot_ps"][:, qlo:],
                        lhsT=v_sb[:, j, :],
                        rhs=gs["pt_g"][:, j, qlo:],
                        start=(tl["first"] and idx == 0),
                        stop=(tl["last"] and idx == nch - 1),
                    )
                if tl["last"]:
                    _emit_epilogue(tl)

            def _emit_epilogue(tl):
                pair, g, nj = tl["pair"], tl["g"], tl["nj"]
                gs = gstate.pop((pair, g))
                ot_ps, acc = gs["ot_ps"], gs["acc"]
                # rest of the epilogue entirely on the Pool queue
                rb = rb_pool.tile([D, 512], F32)
                nc.gpsimd.partition_all_reduce(
                    rb[:], acc[:], channels=128, reduce_op=bass_isa.ReduceOp.add
                )
                if pair not in osbs:
                    osbs[pair] = osb_pool.tile([D, NG, 512], F16, name="out_sb")
                    done_groups[pair] = 0
                out_sb = osbs[pair]
                # the Pool engine only runs its custom ops on real HW, so
                # the normalize lives on DVE: ~51-ULP reciprocal + multiply
                # (the mul reads O^T straight from PSUM, one PSUM operand)
                rcp = o32_pool.tile([D, 512], F32)
                with nc.allow_low_precision(reason="~51 ULP recip"):
                    nc.vector.reciprocal_approx_fast(rcp[:], rb[:])
                nc.vector.tensor_mul(out_sb[:, g], ot_ps[:], rcp[:])
                done_groups[pair] += 1
                if pair == PAIRS - 1:
                    nc.gpsimd.dma_start(ot[pair, g], out_sb[:, g])
                elif done_groups[pair] == NG:
                    # one fused output DMA per pair
                    nc.gpsimd.dma_start(
                        ot[pair].rearrange("g d s -> d g s"), out_sb[:]
                    )

            # software pipeline on the PE queue, continuous across group and
            # pair boundaries, with PV lagging TWO tiles so a PV blocked on
            # its exp never delays the next QK: QK(i) QK(i+1) PV(i-1) ...
            PVLAG = 4
            for i, tl in enumerate(stream):
                if i in prefetch_at:
                    _load_inputs(prefetch_at[i])
                _emit_qk(tl)
                _emit_exp(tl)
                if tl["last"]:
                    _emit_fold(tl)
                if i >= PVLAG:
                    _emit_pv(stream[i - PVLAG])
            for i in range(len(stream) - PVLAG, len(stream)):
                _emit_pv(stream[i])

    nc.compile()
    return nc


_NC = {}


def _get_nc(uniform_mask: bool = True):
    if uniform_mask not in _NC:
        _NC[uniform_mask] = build_module(uniform_mask)
    return _NC[uniform_mask]


def shard_inputs(q, kv, key_padding_mask):
    """Full inputs -> list of 8 per-core input maps (fp16 on device)."""
    q = np.asarray(q)
    kv = np.asarray(kv)
    mask = np.asarray(key_padding_mask)

    pbias = np.where(mask, np.float32(0.0), np.float32(NEG)).astype(np.float32)

    # in-tile causal triangle bias [k, q]: 0 if k <= q else -1e4, plus identity
    kk = np.arange(128)[:, None]
    qq = np.arange(128)[None, :]
    tri_blk = np.where(kk <= qq, np.float32(0.0), np.float32(NEG))
    tri = np.concatenate(
        [
            np.full((128, 384), NEG, np.float32),
            tri_blk,
            np.eye(128, dtype=np.float32),
        ],
        axis=1,
    ).astype(np.float16)

    in_maps = []
    for c in range(N_CORES):
        qc = q[:, :, HPC * c : HPC * (c + 1), :]  # [B, S, 4, D]
        qtc = (
            np.ascontiguousarray(np.transpose(qc, (0, 2, 3, 1)))
            .reshape(PAIRS, D, S)
            .astype(np.float16)
        )
        kc = kv[:, :, 0, c, :]  # [B, S, D]
        vc = kv[:, :, 1, c, :]  # [B, S, D]
        ktc = np.ascontiguousarray(np.transpose(kc, (0, 2, 1))).astype(np.float16)
        in_maps.append(
            {
                "qt": qtc,
                "kt": ktc,
                "v": np.ascontiguousarray(vc).astype(np.float16),
                "tri": tri,
                "pb": pbias,
            }
        )
    return in_maps


def unshard_output(results):
    """Per-core 'ot' [PAIRS, NG, D, 512] fp16 -> full [B, S, H, D] fp32."""
    out = np.empty((B, S, H, D), dtype=np.float32)
    for c in range(N_CORES):
        otc = results[c]["ot"]  # [8, 4, 128, 512]
        for pair in range(PAIRS):
            b, h = pair // HPC, HPC * c + pair % HPC
            out[b, :, h, :] = (
                np.transpose(otc[pair], (0, 2, 1)).reshape(S, D).astype(np.float32)
            )
    return out


def kernel(q, kv, key_padding_mask):
    uniform = bool(np.asarray(key_padding_mask).all())
    nc = _get_nc(uniform)
    in_maps = shard_inputs(q, kv, key_padding_mask)
    res = run_bass_kernel_spmd(nc, in_maps, core_ids=list(range(N_CORES)))
    return unshard_output(res.results)


# revision 29
# speedup vs baseline: 1.0046x; 1.0046x over previous
"""Trainium2 Bass kernel: causal GQA attention (fp16 pipeline).

Problem: B=2, Sq=Sk=2048, H=32, Hkv=8, D=128, fp32 in/out, causal +
key-padding mask.

Sharding (8 cores): head-parallel. Core c takes q-heads [4c, 4c+4) for both
batches; those 4 heads share exactly one kv head (c) per batch, so each core
runs 8 independent (batch, head) pairs — K/V loaded once per batch, no comms.

All device data is fp16 (host converts): matmuls run at 1 PE cycle/row with
no minimum free-dim width, DVE elementwise ops get the 2-byte perf modes,
and DMA bytes halve. Measured numerics: ~4e-4 rel err vs the fp32 reference
(tolerance 2e-2).

Per (batch, head) pair the scores are built TRANSPOSED (keys on partitions,
queries on free) in 512-query groups, key chunks of 128 in 3-chunk ST tiles
(PSUM [128, 3, 512] = 3 banks x 2 bufs + 2 O^T banks = 8 banks). Strict
engine separation so the bottleneck ACT engine never waits on anything but
the PE:

  PE  : S^T[j] = K_j @ Q_g^T, exact causal trimming (live cols only);
        diag chunks += I.T @ tri (128-wide fp16 bias matmul);
        O^T += V_j^T @ P^T[j] accumulated over the group.
        Order is software-pipelined: QK(t0) QK(t1) PV(t0) QK(t2) PV(t1)...
  ACT : P^T = exp(scale*S^T) per ST tile, one fused instruction into a
        per-group P tile [128, 16, 512] fp16.
  DVE : zero the stale cols of diagonal chunks (memset), then one pairwise
        fold-tree per group (tensor_tensor adds at the 2x 2-byte rate over
        from-0 slices) producing acc[128,512] = sum of P over the group's
        key chunks.
  Pool: rb = partition_all_reduce(acc) (cross-partition softmax denom),
        out = O^T / rb (single tensor_tensor divide, PSUM read), then the
        output DMA on the same queue — no cross-engine ping-pong.
  SP  : all input DMAs (sync ring).

The key-padding mask folds into the exp bias per key chunk (bias operand
indexes partitions = keys). The all-ones-mask fast path (the spec's fill)
uses fused tile exps; a non-trivial mask falls back to per-chunk biases.
"""

import math
import sys

import numpy as np

for _p in ("/opt/trn_rl_repo",):
    if _p not in sys.path:
        sys.path.append(_p)

import concourse.bass as bass
import concourse.tile as tile
from concourse import bacc, bass_isa, mybir
from concourse.bass import ts
from concourse.bass_utils import run_bass_kernel_spmd

B = 2
S = 2048
H = 32
HKV = 8
D = 128
N_CORES = 8
HPC = H // N_CORES  # q heads per core = 4
PAIRS = B * HPC  # 8 (batch, head) pairs per core
NG = S // 512  # 4 q-groups of 512 per pair
NCHUNK = S // 128  # 16 key chunks of 128
SCALE = 1.0 / math.sqrt(D)
NEG = -10000.0
TCH = 2  # key chunks per ST tile

# normalize on Pool reading O^T straight from PSUM (False: DVE rcp+mul path)
POOL_DIV = True

F32 = mybir.dt.float32
F16 = mybir.dt.float16
EXP = mybir.ActivationFunctionType.Exp
ADD = mybir.AluOpType.add
DIVIDE = mybir.AluOpType.divide


def build_module(uniform_mask: bool = True):
    nc = bacc.Bacc("TRN2", target_bir_lowering=False, debug=False, num_devices=1)

    qt = nc.dram_tensor("qt", [PAIRS, D, S], F16, kind="ExternalInput").ap()
    kt = nc.dram_tensor("kt", [B, D, S], F16, kind="ExternalInput").ap()
    v = nc.dram_tensor("v", [B, S, D], F16, kind="ExternalInput").ap()
    tri = nc.dram_tensor("tri", [D, 640], F16, kind="ExternalInput").ap()
    pb = nc.dram_tensor("pb", [B, S], F32, kind="ExternalInput").ap()
    ot = nc.dram_tensor("ot", [PAIRS, NG, D, 512], F16, kind="ExternalOutput").ap()

    with tile.TileContext(nc) as tc:
        with (
            tc.tile_pool(name="consts", bufs=1) as consts,
            tc.tile_pool(name="kv", bufs=2) as kv_pool,
            tc.tile_pool(name="q", bufs=4) as q_pool,
            tc.tile_pool(name="ptg", bufs=3) as ptg_pool,
            tc.tile_pool(name="fold", bufs=2) as fold_pool,
            tc.tile_pool(name="acc", bufs=3) as acc_pool,
            tc.tile_pool(name="rb", bufs=3) as rb_pool,
            tc.tile_pool(name="osb", bufs=2) as osb_pool,
            tc.tile_pool(name="o32", bufs=3) as o32_pool,
            tc.tile_pool(name="st_ps", bufs=3, space="PSUM") as st_pool,
            tc.tile_pool(name="ot_ps", bufs=2, space="PSUM") as ot_pool,
        ):
            trid_sb = consts.tile([D, 640], F16)
            negtri_sb = trid_sb[:, :512]  # [neg | tri]: col 512-w.. masks w cols
            ident_sb = trid_sb[:, 512:]
            # warm the ACT exp table during the initial DMAs
            warm_in = consts.tile([1, 2], F32)
            nc.vector.memset(warm_in[:], 1.0)
            warm = consts.tile([1, 2], F32)
            nc.scalar.activation(warm[:], warm_in[:], EXP, scale=1.0)
            for _i in range(3):
                st = st_pool.tile([D, TCH, 512], F32)
                nc.vector.memset(st[:], 0.0)

            kvs = {}
            qts = {}

            def _load_inputs(pairs):
                """Issue input DMAs. Few, large transfers: every DMA burns a
                HWDGE gen slot (625ns, serial device) and a completion-sem
                slot (8 HW slots; reuse blocks until the prior user's waiters
                ran, which throttles prefetch depth). Pair 0 splits its first
                slices out so compute starts ASAP."""
                for p in pairs:
                    b = p // HPC
                    first = p == 0
                    if first:
                        # ordered for fastest time-to-first-matmul
                        kt_sb = kv_pool.tile([D, S], F16, tag="kt")
                        v_r = v[b].rearrange("(j k) d -> k j d", k=128)
                        v_sb = kv_pool.tile([D, NCHUNK, D], F16, tag="v")
                        pb_sb = kv_pool.tile([D, NCHUNK], F32, tag="pb")
                        kvs[b] = (kt_sb, v_sb, pb_sb)
                        qt_sb = q_pool.tile([D, S], F16, tag="qt")
                        qts[p] = qt_sb
                        nc.sync.dma_start(kt_sb[:, ts(0, 512)], kt[b][:, ts(0, 512)])
                        nc.sync.dma_start(qt_sb[:, ts(0, 512)], qt[p][:, ts(0, 512)])
                        nc.sync.dma_start(trid_sb[:], tri[:])
                        nc.sync.dma_start(v_sb[:, ts(0, 4), :], v_r[:, ts(0, 4), :])
                        nc.sync.dma_start(kt_sb[:, ts(1, 512)], kt[b][:, ts(1, 512)])
                        nc.sync.dma_start(qt_sb[:, ts(1, 512)], qt[p][:, ts(1, 512)])
                        nc.sync.dma_start(kt_sb[:, 1024:], kt[b][:, 1024:])
                        nc.sync.dma_start(qt_sb[:, 1024:], qt[p][:, 1024:])
                        nc.sync.dma_start(v_sb[:, 4:, :], v_r[:, 4:, :])
                        if not uniform_mask:
                            nc.sync.dma_start(
                                pb_sb[:], pb[b].rearrange("(j k) -> k j", k=128)
                            )
                        continue
                    if p % HPC == 0 and b not in kvs:
                        kt_sb = kv_pool.tile([D, S], F16, tag="kt")
                        v_r = v[b].rearrange("(j k) d -> k j d", k=128)
                        v_sb = kv_pool.tile([D, NCHUNK, D], F16, tag="v")
                        pb_sb = kv_pool.tile([D, NCHUNK], F32, tag="pb")
                        kvs[b] = (kt_sb, v_sb, pb_sb)
                        nc.sync.dma_start(kt_sb[:], kt[b][:])
                        nc.sync.dma_start(v_sb[:], v_r[:])
                        if not uniform_mask:
                            nc.sync.dma_start(
                                pb_sb[:], pb[b].rearrange("(j k) -> k j", k=128)
                            )
                    qt_sb = q_pool.tile([D, S], F16, tag="qt")
                    qts[p] = qt_sb
                    nc.sync.dma_start(qt_sb[:], qt[p][:])

            # global tile stream: smallest group first on the first pair
            # (fast ramp), largest first on the last pair (short drain tail)
            stream = []
            for pair in range(PAIRS):
                groups = (
                    list(reversed(range(NG))) if pair == PAIRS - 1 else range(NG)
                )
                for g in groups:
                    nj = 4 * (g + 1)
                    ntile = (nj + TCH - 1) // TCH
                    tiles_g = [
                        list(range(TCH * t, min(TCH * t + TCH, nj)))
                        for t in range(ntile)
                    ]
                    # emit the diagonal tiles first and a full-width tile
                    # last, so the group's final exp is big enough to cover
                    # the next group's QK latency (the first emitted tile
                    # still starts at a qlo=0 chunk for the PSUM reset)
                    d0 = (4 * g) // TCH
                    tiles_g = tiles_g[d0:] + tiles_g[:d0]
                    for t, chunks in enumerate(tiles_g):
                        stream.append(
                            dict(
                                pair=pair,
                                g=g,
                                nj=nj,
                                chunks=chunks,
                                qlos=[max(0, 128 * (j - 4 * g)) for j in chunks],
                                first=(t == 0),
                                last=(t == ntile - 1),
                            )
                        )
            # prefetch points: issue pair p's qt (and batch kv) when the
            # previous pair enters its last group
            prefetch_at = {0: [0]}
            for i, tl in enumerate(stream):
                if tl["pair"] > 0 and (
                    i == 0 or stream[i - 1]["pair"] != tl["pair"]
                ):
                    # find start of previous pair's last group
                    j = i - 1
                    while j > 0 and stream[j - 1]["g"] == stream[i - 1]["g"] and (
                        stream[j - 1]["pair"] == stream[i - 1]["pair"]
                    ):
                        j -= 1
                    prefetch_at.setdefault(j, []).append(tl["pair"])

            gstate = {}  # (pair, g) -> dict(ot_ps, pt_g)
            osbs = {}  # pair -> out_sb tile [D, NG, 512]
            done_groups = {}  # pair -> count of completed groups

            def _emit_qk(tl):
                pair, g = tl["pair"], tl["g"]
                kt_sb = kvs[pair // HPC][0]
                qt_sb = qts[pair]
                if (pair, g) not in gstate:
                    gstate[(pair, g)] = dict(
                        ot_ps=ot_pool.tile([D, 512], F32, name="ot_acc"),
                        pt_g=ptg_pool.tile([D, NCHUNK, 512], F16, name="pt_g"),
                    )
                st = st_pool.tile([D, TCH, 512], F32)
                tl["st"] = st
                qmin = min(tl["qlos"])
                for idx, j in enumerate(tl["chunks"]):
                    u = j - 4 * g
                    qlo = tl["qlos"][idx]
                    nc.tensor.matmul(
                        st[:, idx, qlo:],
                        lhsT=kt_sb[:, ts(j, 128)],
                        rhs=qt_sb[:, g * 512 + qlo : (g + 1) * 512],
                        start=True,
                        stop=(u < 0),
                    )
                # mask matmuls after all plain QKs: keeps the tri-const DMA
                # off the first-QK critical path (PSUM adds commute)
                for idx, j in enumerate(tl["chunks"]):
                    u = j - 4 * g
                    qlo = tl["qlos"][idx]
                    if u >= 0:
                        # causal bias on the PE itself; extended down to qmin
                        # so the fused exp writes exact zeros over the stale
                        # sub-diagonal cols (no DVE cleanup needed)
                        w = qlo + 128 - qmin
                        nc.tensor.matmul(
                            st[:, idx, qmin : qlo + 128],
                            lhsT=ident_sb[:],
                            rhs=negtri_sb[:, 512 - w :],
                            start=False,
                            stop=True,
                        )

            def _emit_exp(tl):
                pair, g = tl["pair"], tl["g"]
                pt_g = gstate[(pair, g)]["pt_g"]
                pb_sb = kvs[pair // HPC][2]
                chunks, st = tl["chunks"], tl["st"]
                nch = len(chunks)
                j0 = chunks[0]
                if uniform_mask:
                    qmin = min(tl["qlos"])
                    nc.scalar.activation(
                        pt_g[:, j0 : j0 + nch, qmin:],
                        st[:, :nch, qmin:],
                        EXP,
                        scale=SCALE,
                    )
                else:
                    for idx, j in enumerate(chunks):
                        qlo = tl["qlos"][idx]
                        nc.scalar.activation(
                            pt_g[:, j, qlo:],
                            st[:, idx, qlo:],
                            EXP,
                            bias=pb_sb[:, j : j + 1],
                            scale=SCALE,
                        )
                # zero the cols below qmin that this tile's exp never writes
                # (the extended mask only yields exp=0 down to qmin) so the
                # full-width fold-tree sums exact zeros there
                for idx, j in enumerate(chunks):
                    qlo = qmin if uniform_mask else tl["qlos"][idx]
                    if qlo > 0:
                        nc.vector.memset(pt_g[:, j, :qlo], 0.0)

            def _emit_fold(tl):
                # pairwise fold-tree on DVE: acc = sum of P over the group's
                # chunks. Depends only on the exps, so it is emitted right
                # after the group's last exp to keep the DVE queue flowing.
                pair, g, nj = tl["pair"], tl["g"], tl["nj"]
                gs = gstate[(pair, g)]
                acc = acc_pool.tile([D, 512], F16)
                gs["acc"] = acc
                with nc.allow_low_precision(
                    reason="fp16 softmax denominator, consistent with the "
                    "fp16 P used in PV; pairwise tree, ~1e-3 rel"
                ):
                    n = nj
                    src = gs["pt_g"]
                    while n > 1:
                        half = n // 2
                        if n == 2:
                            nc.vector.tensor_tensor(
                                acc[:], src[:, 0], src[:, 1], ADD
                            )
                            n = 1
                        elif n % 2 == 0:
                            dst = fold_pool.tile(
                                [D, half, 512], F16, tag=f"f{half}"
                            )
                            nc.vector.tensor_tensor(
                                dst[:], src[:, :half], src[:, half : 2 * half], ADD
                            )
                            src, n = dst, half
                        else:  # n == 3
                            dst = fold_pool.tile([D, 512], F16, tag="f1")
                            nc.vector.tensor_tensor(
                                dst[:], src[:, 0], src[:, 1], ADD
                            )
                            nc.vector.tensor_tensor(acc[:], dst[:], src[:, 2], ADD)
                            n = 1

            def _emit_pv(tl):
                pair, g, nj = tl["pair"], tl["g"], tl["nj"]
                gs = gstate[(pair, g)]
                v_sb = kvs[pair // HPC][1]
                nch = len(tl["chunks"])
                for idx, j in enumerate(tl["chunks"]):
                    qlo = tl["qlos"][idx]
                    nc.tensor.matmul(
                        gs["ot_ps"][:, qlo:],
                        lhsT=v_sb[:, j, :],
                        rhs=gs["pt_g"][:, j, qlo:],
                        start=(tl["first"] and idx == 0),
                        stop=(tl["last"] and idx == nch - 1),
                    )
                if tl["last"]:
                    _emit_epilogue(tl)

            def _emit_epilogue(tl):
                pair, g, nj = tl["pair"], tl["g"], tl["nj"]
                gs = gstate.pop((pair, g))
                ot_ps, acc = gs["ot_ps"], gs["acc"]
                # rest of the epilogue entirely on the Pool queue
                rb = rb_pool.tile([D, 512], F32)
                nc.gpsimd.partition_all_reduce(
                    rb[:], acc[:], channels=128, reduce_op=bass_isa.ReduceOp.add
                )
                if pair not in osbs:
                    osbs[pair] = osb_pool.tile([D, NG, 512], F16, name="out_sb")
                    done_groups[pair] = 0
                out_sb = osbs[pair]
                # the Pool engine only runs its custom ops on real HW, so
                # the normalize lives on DVE: ~51-ULP reciprocal + multiply
                # (the mul reads O^T straight from PSUM, one PSUM operand)
                rcp = o32_pool.tile([D, 512], F32)
                with nc.allow_low_precision(reason="~51 ULP recip"):
                    nc.vector.reciprocal_approx_fast(rcp[:], rb[:])
                nc.vector.tensor_mul(out_sb[:, g], ot_ps[:], rcp[:])
                done_groups[pair] += 1
                if pair == PAIRS - 1:
                    # SP ring: idle at the tail, so the gen runs immediately
                    nc.sync.dma_start(ot[pair, g], out_sb[:, g])
                elif done_groups[pair] == NG:
                    # one fused output DMA per pair
                    nc.gpsimd.dma_start(
                        ot[pair].rearrange("g d s -> d g s"), out_sb[:]
                    )

            # software pipeline on the PE queue, continuous across group and
            # pair boundaries, with PV lagging TWO tiles so a PV blocked on
            # its exp never delays the next QK: QK(i) QK(i+1) PV(i-1) ...
            PVLAG = 4
            for i, tl in enumerate(stream):
                if i in prefetch_at:
                    _load_inputs(prefetch_at[i])
                _emit_qk(tl)
                _emit_exp(tl)
                if tl["last"]:
                    _emit_fold(tl)
                if i >= PVLAG:
                    _emit_pv(stream[i - PVLAG])
            for i in range(len(stream) - PVLAG, len(stream)):
                _emit_pv(stream[i])

    nc.compile()
    return nc


_NC = {}


def _get_nc(uniform_mask: bool = True):
    if uniform_mask not in _NC:
        _NC[uniform_mask] = build_module(uniform_mask)
    return _NC[uniform_mask]


def shard_inputs(q, kv, key_padding_mask):
    """Full inputs -> list of 8 per-core input maps (fp16 on device)."""
    q = np.asarray(q)
    kv = np.asarray(kv)
    mask = np.asarray(key_padding_mask)

    pbias = np.where(mask, np.float32(0.0), np.float32(NEG)).astype(np.float32)

    # in-tile causal triangle bias [k, q]: 0 if k <= q else -1e4, plus identity
    kk = np.arange(128)[:, None]
    qq = np.arange(128)[None, :]
    tri_blk = np.where(kk <= qq, np.float32(0.0), np.float32(NEG))
    tri = np.concatenate(
        [
            np.full((128, 384), NEG, np.float32),
            tri_blk,
            np.eye(128, dtype=np.float32),
        ],
        axis=1,
    ).astype(np.float16)

    in_maps = []
    for c in range(N_CORES):
        qc = q[:, :, HPC * c : HPC * (c + 1), :]  # [B, S, 4, D]
        qtc = (
            np.ascontiguousarray(np.transpose(qc, (0, 2, 3, 1)))
            .reshape(PAIRS, D, S)
            .astype(np.float16)
        )
        kc = kv[:, :, 0, c, :]  # [B, S, D]
        vc = kv[:, :, 1, c, :]  # [B, S, D]
        ktc = np.ascontiguousarray(np.transpose(kc, (0, 2, 1))).astype(np.float16)
        in_maps.append(
            {
                "qt": qtc,
                "kt": ktc,
                "v": np.ascontiguousarray(vc).astype(np.float16),
                "tri": tri,
                "pb": pbias,
            }
        )
    return in_maps


def unshard_output(results):
    """Per-core 'ot' [PAIRS, NG, D, 512] fp16 -> full [B, S, H, D] fp32."""
    out = np.empty((B, S, H, D), dtype=np.float32)
    for c in range(N_CORES):
        otc = results[c]["ot"]  # [8, 4, 128, 512]
        for pair in range(PAIRS):
            b, h = pair // HPC, HPC * c + pair % HPC
            out[b, :, h, :] = (
                np.transpose(otc[pair], (0, 2, 1)).reshape(S, D).astype(np.float32)
            )
    return out


def kernel(q, kv, key_padding_mask):
    uniform = bool(np.asarray(key_padding_mask).all())
    nc = _get_nc(uniform)
    in_maps = shard_inputs(q, kv, key_padding_mask)
    res = run_bass_kernel_spmd(nc, in_maps, core_ids=list(range(N_CORES)))
    return unshard_output(res.results)
